# revision 1
# baseline (speedup 1.0000x reference)
"""Trainium2 Bass kernel for Bidirectional Temporal Self Attention.

out = x * (g1+g2+g3) where each g_b = sigmoid(rank1-attention(conv1d(mean_CHW(x)))).

Sharding: pure data parallel over batch N (16) across 8 cores (2 each).
Per core: phase A streams all of x computing per-(n,t) means (pure-read phase
— keeping reads and writes segregated preserves HBM throughput), phase B does
the tiny [1,30] conv + rank-1 attention fully on-chip per batch item (no DMAs
on its critical path; B(0) overlaps A(1)'s streaming), phase C streams x again
multiplying by the broadcast per-(n,t) scale. The last tile of each A(n) stays
resident in SBUF and is reused by C(n) (saves 2 of 24 loads). Loads ride the
sync HWDGE ring; stores ride the scalar HWDGE ring so neither blocks the other.
"""
import numpy as np

import concourse.bass as bass
from concourse import bacc
import concourse.tile as tile
from concourse import mybir
from concourse import bass_utils

N, C, T, H, W = 16, 128, 30, 64, 44
HW = H * W                 # 2816
NCORES = 8
NP_ = N // NCORES          # 2 batch items per core
TB = 5                     # t-block per streamed tile
NBLK = T // TB             # 6 blocks per batch item
F32 = mybir.dt.float32
X_AX = mybir.AxisListType.X
MUL = mybir.AluOpType.mult
ADD = mybir.AluOpType.add

WSPECS = [("wq1", 3), ("wk1", 3), ("wv1", 3),
          ("wq2", 5), ("wk2", 5), ("wv2", 5),
          ("wq3", 7), ("wk3", 7), ("wv3", 7)]
BRANCHES = [("wq1", "wk1", "wv1", 3), ("wq2", "wk2", "wv2", 5),
            ("wq3", "wk3", "wv3", 7)]


def _emit_conv(nc, dst, y1, w_sb, k):
    """dst[1,30] = SAME cross-correlation of y1[1,30] with w_sb[1,k] taps."""
    p = (k - 1) // 2
    nc.vector.memset(dst[:], 0.0)
    for m in range(k):
        s = m - p
        lo, hi = max(0, -s), min(T, T - s)
        nc.vector.scalar_tensor_tensor(
            out=dst[:, lo:hi],
            in0=y1[:, lo + s:hi + s],
            scalar=w_sb[:, m:m + 1],
            in1=dst[:, lo:hi],
            op0=MUL,
            op1=ADD,
        )


def build_bass():
    nc = bacc.Bacc("TRN2")
    x = nc.declare_dram_parameter("x", [NP_, C, T, H, W], F32, isOutput=False)
    wh = {name: nc.declare_dram_parameter(name, [1, 1, k], F32, isOutput=False)
          for name, k in WSPECS}
    out = nc.declare_dram_parameter("out", [NP_, C, T, H, W], F32, isOutput=True)

    xv = x[:].rearrange("n c t h w -> n c t (h w)")
    ov = out[:].rearrange("n c t h w -> n c t (h w)")

    with tile.TileContext(nc) as tc:
        with (
            tc.tile_pool(name="data", bufs=3) as data_pool,
            tc.tile_pool(name="small", bufs=1) as small,
            tc.tile_pool(name="psum", bufs=1, space="PSUM") as psum,
            tc.tile_pool(name="psum_s", bufs=2, space="PSUM") as psum_s,
        ):
            # --- constants / weights (SWDGE: keep the HWDGE rings clear) ---
            w_sb = {}
            for name, k in WSPECS:
                wt = small.tile([1, k], F32, tag=f"w_{name}")
                nc.gpsimd.dma_start(wt[:], wh[name][:].rearrange("a b k -> a (b k)"))
                w_sb[name] = wt
            ones128 = small.tile([128, 1], F32, tag="ones128")
            nc.vector.memset(ones128[:], 1.0)
            ones_1x128 = small.tile([1, 128], F32, tag="ones_1x128")
            nc.vector.memset(ones_1x128[:], 1.0)
            ones11 = small.tile([1, 1], F32, tag="ones11")
            nc.vector.memset(ones11[:], 1.0)

            def emit_phase_a(n):
                P_n = small.tile([128, T], F32, tag=f"P{n}")
                retained = None
                for b in range(NBLK):
                    tl = data_pool.tile([C, TB, HW], F32, tag="data")
                    nc.sync.dma_start(tl[:], xv[n, :, b * TB:(b + 1) * TB, :])
                    nc.vector.reduce_sum(P_n[:, b * TB:(b + 1) * TB], tl[:],
                                         axis=X_AX)
                    if b == NBLK - 1:
                        retained = tl
                return P_n, retained

            def emit_phase_b(n, P_n):
                """Tiny conv + rank-1 attention, all on-chip. Returns scales."""
                y_psum = psum.tile([1, T], F32, tag="y_psum")
                nc.tensor.matmul(y_psum[:], lhsT=ones128[:], rhs=P_n[:],
                                 start=True, stop=True)
                y1 = small.tile([1, T], F32, tag=f"y{n}")
                nc.scalar.mul(y1[:], y_psum[:], 1.0 / float(C * HW))

                gsum = small.tile([1, T], F32, tag=f"gsum{n}")
                for bi, (qn, kn, vn, ksz) in enumerate(BRANCHES):
                    q_t = small.tile([1, T], F32, tag=f"q{n}_{bi}")
                    k_t = small.tile([1, T], F32, tag=f"k{n}_{bi}")
                    v_t = small.tile([1, T], F32, tag=f"v{n}_{bi}")
                    _emit_conv(nc, q_t, y1, w_sb[qn], ksz)
                    _emit_conv(nc, k_t, y1, w_sb[kn], ksz)
                    _emit_conv(nc, v_t, y1, w_sb[vn], ksz)

                    # S[i,t] = q[i] * k[t]  (rank-1 outer product)
                    S = psum_s.tile([T, T], F32, tag="S")
                    nc.tensor.matmul(S[:], lhsT=q_t[:], rhs=k_t[:],
                                     start=True, stop=True)
                    mx = small.tile([T, 1], F32, tag=f"mx{n}_{bi}")
                    nc.vector.reduce_max(mx[:], S[:], axis=X_AX)
                    nmx = small.tile([T, 1], F32, tag=f"nmx{n}_{bi}")
                    nc.vector.tensor_scalar_mul(nmx[:], mx[:], -1.0)
                    E = small.tile([T, T], F32, tag=f"E{n}_{bi}")
                    nc.scalar.activation(E[:], S[:],
                                         mybir.ActivationFunctionType.Exp,
                                         bias=nmx[:], scale=1.0)
                    Z = small.tile([T, 1], F32, tag=f"Z{n}_{bi}")
                    nc.vector.reduce_sum(Z[:], E[:], axis=X_AX)
                    R = small.tile([T, 1], F32, tag=f"R{n}_{bi}")
                    nc.vector.reciprocal(R[:], Z[:])
                    # v as a column vector via K=1 matmul (v^T @ [1])
                    vT = psum_s.tile([T, 1], F32, tag="vT")
                    nc.tensor.matmul(vT[:], lhsT=v_t[:], rhs=ones11[:],
                                     start=True, stop=True)
                    c_t = small.tile([T, 1], F32, tag=f"c{n}_{bi}")
                    nc.vector.tensor_mul(c_t[:], vT[:], R[:])
                    # out[t] = sum_i c[i] * E[i,t]
                    outp = psum.tile([1, T], F32, tag="outp")
                    nc.tensor.matmul(outp[:], lhsT=c_t[:], rhs=E[:],
                                     start=True, stop=True)
                    if bi == 0:
                        nc.scalar.activation(gsum[:], outp[:],
                                             mybir.ActivationFunctionType.Sigmoid)
                    else:
                        g_b = small.tile([1, T], F32, tag=f"g{n}_{bi}")
                        nc.scalar.activation(g_b[:], outp[:],
                                             mybir.ActivationFunctionType.Sigmoid)
                        nc.vector.tensor_add(gsum[:], gsum[:], g_b[:])

                # broadcast gsum to all 128 partitions
                sc_psum = psum.tile([C, T], F32, tag="sc_psum")
                nc.tensor.matmul(sc_psum[:], lhsT=ones_1x128[:], rhs=gsum[:],
                                 start=True, stop=True)
                scales = small.tile([C, T], F32, tag=f"scales{n}")
                nc.vector.tensor_copy(scales[:], sc_psum[:])
                return scales

            def emit_phase_c(n, scales, retained):
                order = [NBLK - 1] + list(range(NBLK - 1))
                for b in order:
                    if b == NBLK - 1:
                        tl = retained
                    else:
                        tl = data_pool.tile([C, TB, HW], F32, tag="data")
                        nc.sync.dma_start(tl[:], xv[n, :, b * TB:(b + 1) * TB, :])
                    for i in range(TB):
                        nc.vector.tensor_scalar_mul(
                            tl[:, i, :], tl[:, i, :],
                            scales[:, b * TB + i:b * TB + i + 1])
                    nc.scalar.dma_start(ov[n, :, b * TB:(b + 1) * TB, :], tl[:])

            # Global ordering: all loads first (long pure-read phase), B(0)
            # overlaps A(1) streaming, C after.
            P0, ret0 = emit_phase_a(0)
            scales0 = emit_phase_b(0, P0)
            P1, ret1 = emit_phase_a(1)
            scales1 = emit_phase_b(1, P1)
            emit_phase_c(0, scales0, ret0)
            emit_phase_c(1, scales1, ret1)

    nc.compile()
    return nc


_NC_CACHE = None


def _get_nc():
    global _NC_CACHE
    if _NC_CACHE is None:
        _NC_CACHE = build_bass()
    return _NC_CACHE


def run(inputs, trace=False, **kw):
    nc = _get_nc()
    x = np.ascontiguousarray(inputs["x"], dtype=np.float32)
    assert x.shape == (N, C, T, H, W), x.shape
    ws = {name: np.ascontiguousarray(inputs[name], dtype=np.float32)
          for name, _ in WSPECS}
    in_maps = []
    for c in range(NCORES):
        m = {"x": x[NP_ * c:NP_ * (c + 1)]}
        m.update(ws)
        in_maps.append(m)
    res = bass_utils.run_bass_kernel_spmd(
        nc, in_maps, core_ids=list(range(NCORES)), trace=trace, **kw)
    outs = np.concatenate([r["out"] for r in res.results], axis=0)
    return outs, res


def kernel(**inputs) -> np.ndarray:
    outs, _ = run(inputs, trace=False)
    return outs

